# revision 76
# baseline (speedup 1.0000x reference)
"""Trainium2 Bass kernel for nn_Decoder (2-layer LSTM decoder with BatchNorm +
LockedDropout + vocab projection), tensor-parallel over the hidden dim across
8 NeuronCores.

Contract: kernel(**inputs) takes FULL inputs (as produced by setup_inputs())
and returns the FULL [B*T, V] float32 output.

Sharding:
  - Each core owns a 128-unit slice of the hidden dim for BOTH LSTM layers
    (gates i,f,g,o for those units) -> gate matmuls have M=128 per gate with
    full batch B=256 as the moving dim (full PE width, BN stats exact).
  - Recurrent state h1/h2 is all-gathered across cores every step (ncfw
    AllGather through HBM).  y1 rides with h1; y2 rides with h2.
  - Layer 2 is software-pipelined one step behind layer 1 (iteration `it`
    computes l1(it) and l2(it-1)), so every matmul in iteration `it` only
    consumes collectives issued in iteration `it-1` -- a full ~21us of PE
    work covers each AllGather round-trip and the PE never starves (which
    also keeps the PE p-state ramped).
  - The vocab projection is sharded over V (1250 per core) and lags two
    steps (iteration `it` projects y2(it-2)) as guaranteed-ready PE filler.
  - Matmuls and all gathered activations run in bfloat16 (full PE rate,
    half the HBM/wire traffic of fp32r -- the DMA engines are the binding
    resource otherwise).  Weights and x are pre-cast on the host; on-chip
    producers write bf16 directly.  Measured end-to-end max-rel error vs
    the fp32 reference: ~1.2e-2 (threshold 2e-2).
"""

import contextlib
import os
import sys

sys.path.insert(0, "/opt/trn_rl_repo")

import ml_dtypes
import numpy as np

import concourse.bass as bass
import concourse.tile as tile
from concourse import bacc, mybir
from concourse.bass_utils import run_bass_kernel_spmd

F32 = mybir.dt.float32
F32R = mybir.dt.float32r
BF16 = mybir.dt.bfloat16

# matmul/transport dtype: "f32r" (full-rate, near-fp32), "bf16", "f32" (4x slower)
DT_MM_NAME = os.environ.get("TRN_DT_MM", "bf16")
DT_MM = {"f32r": F32R, "bf16": BF16, "f32": F32}[DT_MM_NAME]

B, L, E, H, V = 256, 20, 512, 1024, 10000
T = int(os.environ.get("TRN_T", L + 1))
NCORE = 8
P = 128
HS = H // NCORE          # 128 hidden units per core per layer
VS = V // NCORE          # 1250 vocab slots per core
NKE = E // P             # 4 k-tiles over E
NKH = H // P             # 8 k-tiles over H
BN_EPS = 1e-5
# projection N-chunks of VS=1250 (each >=256 so fp32r runs full rate;
# fp32r requires even N and 8-byte-aligned dst start)
NCHUNKS = [(0, 418), (418, 416), (834, 416)]

LAST_EXEC_NS = None
# queue for gather-read DMAs: sp | act | pool
READQ = os.environ.get("TRN_READQ", "sp")
# TRN_FAKE_AG=1 replaces collectives with a local DMA (timing-model runs only)
FAKE_AG = os.environ.get("TRN_FAKE_AG", "0") == "1"
# store projection results straight from PSUM (no DVE staging copy)
PSUM_DMA = os.environ.get("TRN_PSUM_DMA", "1") == "1"

_CACHE = {}


def _fp32r_round(x):
    """Round fp32 -> nearest fp32r value (sum of two bf16s), like the
    hardware's rounding producers / walrus cast_fp32_to_fp32r."""
    hi = x.astype(ml_dtypes.bfloat16).astype(np.float32)
    lo = (x - hi).astype(ml_dtypes.bfloat16).astype(np.float32)
    return hi + lo


def build_bass():
    nc = bacc.Bacc("TRN2", target_bir_lowering=False, num_devices=NCORE)
    dt = DT_MM
    cast = False                # host pre-converts; DMA straight into tiles
    # DRAM dtype for weights/x matches the tile dtype (host pre-rounds fp32r
    # / pre-casts bf16).
    ddt = dt if dt in (F32R, BF16) else F32

    # ---------------- DRAM I/O ----------------
    d_xT = nc.dram_tensor("xT", [T, NKE, P, B], ddt, kind="ExternalInput")
    d_wih1 = nc.dram_tensor("wih1", [4, NKE, P, HS], ddt, kind="ExternalInput")
    d_whh1 = nc.dram_tensor("whh1", [4, NKH, P, HS], ddt, kind="ExternalInput")
    d_wih2 = nc.dram_tensor("wih2", [4, NKH, P, HS], ddt, kind="ExternalInput")
    d_whh2 = nc.dram_tensor("whh2", [4, NKH, P, HS], ddt, kind="ExternalInput")
    d_woutT = nc.dram_tensor("woutT", [NKH, P, VS], ddt, kind="ExternalInput")
    d_bias1 = nc.dram_tensor("bias1", [HS, 4], F32, kind="ExternalInput")
    d_bias2 = nc.dram_tensor("bias2", [HS, 4], F32, kind="ExternalInput")
    d_gb1 = nc.dram_tensor("gb1", [HS, 2], F32, kind="ExternalInput")
    d_gb2 = nc.dram_tensor("gb2", [HS, 2], F32, kind="ExternalInput")
    d_m1T = nc.dram_tensor("m1T", [HS, B], F32, kind="ExternalInput")
    d_m2T = nc.dram_tensor("m2T", [HS, B], F32, kind="ExternalInput")
    d_out = nc.dram_tensor("out", [B * T, VS], dt if dt == BF16 else F32,
                           kind="ExternalOutput")
    # out rows are (b, t) packed; view for per-(t, b-block) strided writes
    d_out_r = d_out[:].rearrange("(b t) v -> b t v", t=T)

    # collective bounce buffers (inputs must be Local, outputs Shared)
    RING = 4
    ag1i = [nc.dram_tensor(f"ag1i{j}", [2 * P, B], dt, kind="Internal")
            for j in range(RING)]
    ag1o = [nc.dram_tensor(f"ag1o{j}", [2 * P * NCORE, B], dt,
                           kind="Internal", addr_space="Shared")
            for j in range(RING)]
    ag2i = [nc.dram_tensor(f"ag2i{j}", [2 * P, B], dt, kind="Internal")
            for j in range(RING)]
    ag2o = [nc.dram_tensor(f"ag2o{j}", [2 * P * NCORE, B], dt,
                           kind="Internal", addr_space="Shared")
            for j in range(RING)]

    dma = nc.sync.dma_start
    rdma = {"sp": nc.sync.dma_start, "act": nc.scalar.dma_start,
            "pool": nc.gpsimd.dma_start}[READQ]

    with tile.TileContext(nc) as tc:
        with contextlib.ExitStack() as ctx:
            smalls = ctx.enter_context(tc.tile_pool(name="smalls", bufs=1))
            wts = ctx.enter_context(tc.tile_pool(name="wts", bufs=1))
            stage = ctx.enter_context(tc.tile_pool(name="stage", bufs=2))
            xpool = ctx.enter_context(tc.tile_pool(name="xpool", bufs=3))
            gp_h1 = ctx.enter_context(tc.tile_pool(name="g_h1", bufs=2))
            gp_y1 = ctx.enter_context(tc.tile_pool(name="g_y1", bufs=2))
            gp_h2 = ctx.enter_context(tc.tile_pool(name="g_h2", bufs=2))
            gp_yy = ctx.enter_context(tc.tile_pool(name="g_yy", bufs=2))
            cell = ctx.enter_context(tc.tile_pool(name="cell", bufs=2))
            slpool = ctx.enter_context(tc.tile_pool(name="slp", bufs=2))
            state = ctx.enter_context(tc.tile_pool(name="state", bufs=1))
            psumg = ctx.enter_context(
                tc.tile_pool(name="psumg", bufs=2, space="PSUM"))
            psum2 = ctx.enter_context(
                tc.tile_pool(name="psum2", bufs=2, space="PSUM"))
            psumP = ctx.enter_context(
                tc.tile_pool(name="psumP", bufs=4, space="PSUM"))
            outp = ctx.enter_context(tc.tile_pool(name="outp", bufs=3))

            # small constants (loaded after wih1/x0 below -- they are
            # first needed by cell1(0)'s activations, not the first matmul)
            b1 = smalls.tile([HS, 4], F32)
            b2 = smalls.tile([HS, 4], F32)
            gb1 = smalls.tile([HS, 2], F32)
            gb2 = smalls.tile([HS, 2], F32)
            m1 = smalls.tile([HS, B], F32)
            m2 = smalls.tile([HS, B], F32)

            # resident weights
            w_ih1 = wts.tile([P, 4, NKE, HS], dt)
            w_hh1 = wts.tile([P, 4, NKH, HS], dt)
            w_ih2 = wts.tile([P, 4, NKH, HS], dt)
            w_hh2 = wts.tile([P, 4, NKH, HS], dt)
            w_out = wts.tile([P, NKH, VS], dt)

            def load_weight(dst, dram, n_g, n_k):
                # dst [P, n_g, n_k, HS]; dram [n_g, n_k, P, HS]
                for g in range(n_g):
                    if cast:
                        st = stage.tile([P, n_k, HS], F32, tag="wstage",
                                        name=f"wst_{dram.name}_{g}")
                        dma(st[:], dram[g][:].rearrange("k p m -> p k m"))
                        nc.vector.tensor_copy(dst[:, g], st[:])
                    else:
                        dma(dst[:, g], dram[g][:].rearrange("k p m -> p k m"))

            # wih1 first so ih1(0) can start while the rest stream in; wout
            # last (first needed by proj(0) in iteration 2).
            load_weight(w_ih1, d_wih1, 4, NKE)

            # persistent state
            c1 = state.tile([P, B], F32)
            c2 = state.tile([P, B], F32)
            nc.vector.memset(c1[:], 0.0)
            nc.vector.memset(c2[:], 0.0)

            def lstm_cell(emit_gates, bias, gbv, mask, c_st,
                          h_out, y_out, t, pgs=None, psum_pool=None,
                          y_ctx=None):
                """One LSTM cell + BatchNorm + dropout-mask.

                psum packing: pgA=(i,g), pgB=(f,o); gate order i=0 f=1 g=2 o=3.
                pgs: pre-allocated (pgA, pgB) whose accumulation was already
                started (x-side matmuls emitted in the previous step).
                """
                if pgs is None:
                    pool = psum_pool or psumg
                    pgA = pool.tile([P, 2, B], F32, tag="pg", name=f"pgA_{t}")
                    pgB = pool.tile([P, 2, B], F32, tag="pg", name=f"pgB_{t}")
                else:
                    pgA, pgB = pgs
                gloc = {0: (pgA, 0), 2: (pgA, 1), 1: (pgB, 0), 3: (pgB, 1)}
                for gate in (0, 2, 1, 3):
                    tl, sub = gloc[gate]
                    emit_gates(gate, tl[:, sub])

                i_t = cell.tile([P, B], F32, tag="i", name=f"i_{t}")
                f_t = cell.tile([P, B], F32, tag="f", name=f"f_{t}")
                g_t = cell.tile([P, B], F32, tag="g", name=f"g_{t}")
                o_t = cell.tile([P, B], F32, tag="o", name=f"o_{t}")
                Sig = mybir.ActivationFunctionType.Sigmoid
                Tanh = mybir.ActivationFunctionType.Tanh
                nc.scalar.activation(i_t[:], pgA[:, 0], Sig, bias=bias[:, 0:1])
                nc.scalar.activation(g_t[:], pgA[:, 1], Tanh, bias=bias[:, 2:3])
                nc.scalar.activation(f_t[:], pgB[:, 0], Sig, bias=bias[:, 1:2])
                nc.scalar.activation(o_t[:], pgB[:, 1], Sig, bias=bias[:, 3:4])

                ig = cell.tile([P, B], F32, tag="ig", name=f"ig_{t}")
                nc.vector.tensor_mul(ig[:], i_t[:], g_t[:])
                fc = cell.tile([P, B], F32, tag="fc", name=f"fc_{t}")
                nc.vector.tensor_mul(fc[:], f_t[:], c_st[:])
                nc.vector.tensor_add(c_st[:], ig[:], fc[:])
                tnc = cell.tile([P, B], F32, tag="tc", name=f"tc_{t}")
                nc.scalar.activation(tnc[:], c_st[:], Tanh)
                h_f = cell.tile([P, B], F32, tag="h", name=f"h_{t}")
                nc.vector.tensor_mul(h_f[:], o_t[:], tnc[:])
                if dt != F32:
                    nc.gpsimd.tensor_copy(h_out[:], h_f[:])
                # BN stats over batch (free dim); bn_stats/bn_aggr are
                # DVE-only, but the rsqrt Newton chain, the affine params
                # and the mask-multiply all run on the (otherwise idle)
                # Pool engine so the DVE/ACT tails never gate downstream
                # consumers.
                st6 = cell.tile([P, 6], F32, tag="st", name=f"st_{t}")
                nc.vector.bn_stats(st6[:], h_f[:])
                mv = cell.tile([P, 2], F32, tag="mv", name=f"mv_{t}")
                nc.vector.bn_aggr(mv[:], st6[:])
                I32 = mybir.dt.int32
                v_t = cell.tile([P, 1], F32, tag="vv", name=f"vv_{t}")
                nc.vector.tensor_scalar_add(v_t[:], mv[:, 1:2], BN_EPS)
                r_a = cell.tile([P, 1], F32, tag="ra", name=f"ra_{t}")
                r_b = cell.tile([P, 1], F32, tag="rb", name=f"rb_{t}")
                ui = cell.tile([P, 1], I32, tag="ui", name=f"ui_{t}")
                nc.vector.tensor_scalar(ui[:], v_t[:].bitcast(I32), 1, None,
                                        op0=mybir.AluOpType.logical_shift_right)
                nc.vector.tensor_scalar(r_a[:].bitcast(I32), ui[:],
                                        -1, 0x5F3759DF,
                                        op0=mybir.AluOpType.mult,
                                        op1=mybir.AluOpType.add)
                rr = cell.tile([P, 1], F32, tag="rr", name=f"rr_{t}")
                ww = cell.tile([P, 1], F32, tag="ww", name=f"ww_{t}")
                r_cur, r_nxt = r_a, r_b
                for it in range(2):
                    nc.vector.tensor_mul(rr[:], r_cur[:], r_cur[:])
                    nc.vector.scalar_tensor_tensor(
                        ww[:], rr[:], -0.5, v_t[:],
                        op0=mybir.AluOpType.mult, op1=mybir.AluOpType.mult)
                    nc.vector.scalar_tensor_tensor(
                        r_nxt[:], ww[:], 1.5, r_cur[:],
                        op0=mybir.AluOpType.add, op1=mybir.AluOpType.mult)
                    r_cur, r_nxt = r_nxt, r_cur
                a_v = cell.tile([P, 1], F32, tag="av", name=f"av_{t}")
                nc.vector.tensor_mul(a_v[:], r_cur[:], gbv[:, 0:1])
                ma = cell.tile([P, 1], F32, tag="ma", name=f"ma_{t}")
                nc.vector.tensor_mul(ma[:], mv[:, 0:1], a_v[:])
                b_v = cell.tile([P, 1], F32, tag="bv", name=f"bv_{t}")
                nc.vector.tensor_sub(b_v[:], gbv[:, 1:2], ma[:])
                yt = cell.tile([P, B], F32, tag="yt", name=f"yt_{t}")
                with (y_ctx() if y_ctx is not None
                      else contextlib.nullcontext()):
                    nc.scalar.activation(
                        yt[:], h_f[:],
                        mybir.ActivationFunctionType.Identity,
                        bias=b_v[:], scale=a_v[:])
                    nc.vector.tensor_mul(y_out[:], yt[:], mask[:])
                return h_f

            def project(tp, y2g_src, copy_ctx=None):
                """Projection of step tp's y2 (gathered in y2g_src).

                Chunk-outer / k-inner: each vocab chunk finishes its full
                K accumulation before the next starts, so only 2 PSUM banks
                double-buffer the whole projection (chunk n+1 accumulates
                while chunk n copies out on Pool)."""
                for bh in range(2):
                    lhs = [y2g_src[:, k, bh * P:(bh + 1) * P]
                           for k in range(NKH)]
                    for n, (noff, nlen) in enumerate(NCHUNKS):
                        pp = psumP.tile([P, 512], F32, tag="pp",
                                        name=f"pp_{tp}_{bh}_{n}")
                        for k in range(NKH):
                            nc.tensor.matmul(
                                pp[:, 0:nlen], lhs[k],
                                w_out[:, k, noff:noff + nlen],
                                start=(k == 0), stop=(k == NKH - 1))
                        o_sb = outp.tile([P, 432], dt if dt == BF16 else F32,
                                         tag="osb",
                                         name=f"osb_{tp}_{bh}_{n}")
                        if PSUM_DMA:
                            # PSUM->SBUF copies alternate ACT/DVE (Pool
                            # cannot read PSUM); the DRAM store goes on the
                            # Pool queue so staging recycling never queues
                            # behind the AllGather DMA chain
                            with (copy_ctx() if copy_ctx is not None
                                  else contextlib.nullcontext()):
                                nc.scalar.copy(o_sb[:, 0:nlen],
                                               pp[:, 0:nlen])
                            nc.gpsimd.dma_start(
                                d_out_r[bh * P:(bh + 1) * P, tp,
                                        noff:noff + nlen], o_sb[:, 0:nlen])
                        else:
                            nc.vector.tensor_copy(o_sb[:, 0:nlen],
                                                  pp[:, 0:nlen])
                            dma(d_out_r[bh * P:(bh + 1) * P, tp,
                                        noff:noff + nlen], o_sb[:, 0:nlen])

            def load_x(t):
                x_t = xpool.tile([P, NKE, B], dt, tag="x", name=f"x_{t}")
                if cast:
                    xs = xpool.tile([P, NKE, B], F32, tag="xs", name=f"xs_{t}")
                    dma(xs[:], d_xT[t][:].rearrange("k p b -> p k b"))
                    nc.vector.tensor_copy(x_t[:], xs[:])
                else:
                    dma(x_t[:], d_xT[t][:].rearrange("k p b -> p k b"))
                return x_t

            def emit_ih1(t, x_t, stop):
                # x-side of layer-1 gates for step t.  One accumulation group
                # per PSUM bank: start on the bank's first matmul (gates 0/1),
                # stop on its last (gates 2/3; deferred to the hh matmuls
                # unless `stop`).  Gate g's first matmul uses start=False --
                # its half-bank has_written bits are clear, so it overwrites.
                pgA = psumg.tile([P, 2, B], F32, tag="pg", name=f"pgA_1_{t}")
                pgB = psumg.tile([P, 2, B], F32, tag="pg", name=f"pgB_1_{t}")
                gloc = {0: (pgA, 0), 2: (pgA, 1), 1: (pgB, 0), 3: (pgB, 1)}
                for gate in (0, 2, 1, 3):
                    tl, sub = gloc[gate]
                    for k in range(NKE):
                        nc.tensor.matmul(
                            tl[:, sub], w_ih1[:, gate, k], x_t[:, k],
                            start=(k == 0 and gate in (0, 1)),
                            stop=(stop and k == NKE - 1 and gate in (2, 3)))
                return pgA, pgB

            x_tiles = {0: load_x(0)}
            pending_l1 = emit_ih1(0, x_tiles.pop(0), stop=True)
            x_tiles[1] = load_x(1)
            for sdst, ssrc in ((b1, d_bias1), (b2, d_bias2), (gb1, d_gb1),
                               (gb2, d_gb2), (m1, d_m1T), (m2, d_m2T)):
                dma(sdst[:], ssrc[:])

            # remaining weights stream in behind x(0)/wih1
            load_weight(w_hh1, d_whh1, 4, NKH)
            load_weight(w_ih2, d_wih2, 4, NKH)
            load_weight(w_hh2, d_whh2, 4, NKH)
            for k in range(NKH):
                if cast:
                    st = stage.tile([P, VS], F32, tag="pstage",
                                    name=f"wst_wout_{k}")
                    dma(st[:], d_woutT[k][:])
                    nc.vector.tensor_copy(w_out[:, k], st[:])
                else:
                    dma(w_out[:, k], d_woutT[k][:])

            # Iteration `it` computes l1(it), l2(it-2), proj(it-3); every
            # input was gathered >=1 full iteration earlier (AG1(it)'s
            # consumers hh1(it+1) / ih2 at it+2; AG2's consumers two
            # iterations later), so no PE work ever waits on an in-flight
            # collective.
            h1g_prev = None    # gathered h1(it-1)
            y1g_prev = None    # gathered y1(it-1)
            y1g_prev2 = None   # gathered y1(it-2)
            h2g_prev = None    # gathered h2(it-3)
            y2g_prev = None    # gathered y2(it-3)

            # The Tile scheduler's internal cost model sees near-instant
            # gathers and so orders hh1(it) LAST in each iteration, which
            # serializes cell1 -> AllGather round-trip after the PE has
            # drained all fill work (an ~11us stall per step).  Force the
            # intended static order with logical wait-until slots: hh1+cell1
            # first (so AG1 issues early), then proj / l2 / next-x as fill.
            SLOT_MS = float(os.environ.get("TRN_SLOT_MS", "0.05"))

            def slot(i):
                return tc.tile_wait_until(i * SLOT_MS)

            for it in range(T + 2):
                h1g_cur = y1g_cur = h2g_cur = y2g_cur = None
                sbase = 1 + 4 * it

                # ---- layer 1, step it (x-side already emitted; add hh1) ---
                if it < T:
                    def l1_gates(gate, pap, h1g=h1g_prev, t=it):
                        if t > 0:
                            for k in range(NKH):
                                nc.tensor.matmul(
                                    pap, w_hh1[:, gate, k], h1g[:, k],
                                    start=False,
                                    stop=(k == NKH - 1 and gate in (2, 3)))

                    with slot(sbase + 0):
                        if it > 0:
                            pending_l1 = emit_ih1(it, x_tiles.pop(it),
                                                  stop=False)
                        h1_sl = slpool.tile([P, B], dt, tag="h1s",
                                            name=f"h1s_{it}")
                        y1_sl = slpool.tile([P, B], dt, tag="y1s",
                                            name=f"y1s_{it}")
                        lstm_cell(l1_gates, b1, gb1, m1, c1,
                                  h1_sl, y1_sl, f"1_{it}",
                                  pgs=pending_l1)

                        # ---- AllGather (h1 ; y1) ----
                        agi = ag1i[it % RING]
                        ago1 = ag1o[it % RING]
                        dma(agi[0:P], h1_sl[:])
                        dma(agi[P:2 * P], y1_sl[:])
                        if FAKE_AG:
                            dma(ago1[0:2 * P], agi[:])
                        else:
                            nc.gpsimd.collective_compute(
                                "AllGather", mybir.AluOpType.bypass,
                                ins=[agi[:].opt()], outs=[ago1[:].opt()],
                                replica_groups=[list(range(NCORE))])
                        agor = ago1[:].rearrange("(c two p) b -> two p c b",
                                                 two=2, p=P)
                        h1g_cur = gp_h1.tile([P, NCORE, B], dt, tag="h1g",
                                             name=f"h1g_{it}")
                        dma(h1g_cur[:], agor[0])
                        y1g_cur = gp_y1.tile([P, NCORE, B], dt, tag="y1g",
                                             name=f"y1g_{it}")
                        dma(y1g_cur[:], agor[1])

                # ---- layer 2, step it-1 ----
                if 1 <= it <= T:
                    s = it - 1

                    def l2_gates(gate, pap, y1g=y1g_prev, h2g=h2g_prev,
                                 s=s):
                        if s > 0:
                            for k in range(NKH):
                                nc.tensor.matmul(
                                    pap, w_hh2[:, gate, k], h2g[:, k],
                                    start=(k == 0), stop=False)
                        for k in range(NKH):
                            nc.tensor.matmul(
                                pap, w_ih2[:, gate, k], y1g[:, k],
                                start=(s == 0 and k == 0),
                                stop=(k == NKH - 1))

                    with slot(sbase + 1):
                        h2_sl = slpool.tile([P, B], dt, tag="h2s",
                                            name=f"h2s_{s}")
                        y2_sl = slpool.tile([P, B], dt, tag="y2s",
                                            name=f"y2s_{s}")
                        lstm_cell(l2_gates, b2, gb2, m2, c2,
                                  h2_sl, y2_sl, f"2_{s}",
                                  psum_pool=psum2)

                        # ---- AllGather (h2 ; y2) ----
                        agi = ag2i[s % RING]
                        ago2 = ag2o[s % RING]
                        dma(agi[0:P], h2_sl[:])
                        dma(agi[P:2 * P], y2_sl[:])
                        if FAKE_AG:
                            dma(ago2[0:2 * P], agi[:])
                        else:
                            nc.gpsimd.collective_compute(
                                "AllGather", mybir.AluOpType.bypass,
                                ins=[agi[:].opt()], outs=[ago2[:].opt()],
                                replica_groups=[list(range(NCORE))])

                # ---- h2/y2 gather reads (SP, in-block; separate pools
                # so h2g and y2g recycle independently -- a combined tile
                # couples its write to the slowest consumer) ----
                if 1 <= it <= T:
                    with slot(sbase + 1):
                        agor2 = ago2[:].rearrange("(c two p) b -> two p c b",
                                                  two=2, p=P)
                        h2g_cur = None
                        if it - 1 < T - 1:
                            h2g_cur = gp_h2.tile([P, NCORE, B], dt,
                                                 tag="h2g",
                                                 name=f"h2g_{it - 1}")
                            dma(h2g_cur[:], agor2[0])
                        y2g_cur = gp_yy.tile([P, NCORE, B], dt, tag="y2g",
                                             name=f"y2g_{it - 1}")
                        dma(y2g_cur[:], agor2[1])

                # ---- projection of step it-2 (ready PE filler) ----
                if it >= 2:
                    with slot(sbase + 2):
                        project(it - 2, y2g_prev)

                # ---- prefetch x two iterations ahead so its DMA never
                # gates the PE ----
                with slot(sbase + 3):
                    if it + 2 < T:
                        x_tiles[it + 2] = load_x(it + 2)

                y1g_prev2 = y1g_prev
                if it < T:
                    h1g_prev, y1g_prev = h1g_cur, y1g_cur
                if h2g_cur is not None:
                    h2g_prev = h2g_cur
                if y2g_cur is not None:
                    y2g_prev = y2g_cur

    nc.finalize()
    return nc


def _prep_inputs(features, captions, lengths, embed_table,
                 W_ih1, W_hh1, b_ih1, b_hh1, gamma1, beta1, mask1,
                 W_ih2, W_hh2, b_ih2, b_hh2, gamma2, beta2, mask2,
                 W_out, b_out):
    f32 = np.float32
    features = np.asarray(features, f32)
    captions = np.asarray(captions)
    embed_table = np.asarray(embed_table, f32)
    if DT_MM == F32R:
        rnd = _fp32r_round
    elif DT_MM == BF16:
        rnd = lambda a: np.ascontiguousarray(a.astype(ml_dtypes.bfloat16))
    else:
        rnd = lambda a: a

    # x sequence [T, B, E] -> xT [T, NKE, P, B]
    x = np.empty((L + 1, B, E), f32)
    x[0] = features
    x[1:] = embed_table[captions].transpose(1, 0, 2)
    x = x[:T]
    xT = rnd(np.ascontiguousarray(x.transpose(0, 2, 1).reshape(T, NKE, P, B)))

    def wslice(Wf, c, K):
        # Wf [4H, K] -> per-core [4, K//P, P, HS] lhsT blocks
        Wg = np.asarray(Wf, f32).reshape(4, H, K)[:, c * HS:(c + 1) * HS, :]
        # out[g, k, kk, m] = Wg[g, m, k*P + kk]
        return rnd(np.ascontiguousarray(
            Wg.transpose(0, 2, 1).reshape(4, K // P, P, HS)))

    bsum1 = (np.asarray(b_ih1, f32) + np.asarray(b_hh1, f32)).reshape(4, H)
    bsum2 = (np.asarray(b_ih2, f32) + np.asarray(b_hh2, f32)).reshape(4, H)
    WoT = np.ascontiguousarray(np.asarray(W_out, f32).T)  # [H, V]

    in_maps = []
    for c in range(NCORE):
        u = slice(c * HS, (c + 1) * HS)
        v = slice(c * VS, (c + 1) * VS)
        in_maps.append({
            "xT": xT,
            "wih1": wslice(W_ih1, c, E),
            "whh1": wslice(W_hh1, c, H),
            "wih2": wslice(W_ih2, c, H),
            "whh2": wslice(W_hh2, c, H),
            "woutT": rnd(np.ascontiguousarray(
                WoT[:, v].reshape(NKH, P, VS))),
            "bias1": np.ascontiguousarray(bsum1[:, u].T),
            "bias2": np.ascontiguousarray(bsum2[:, u].T),
            "gb1": np.ascontiguousarray(
                np.stack([np.asarray(gamma1, f32)[u],
                          np.asarray(beta1, f32)[u]], axis=1)),
            "gb2": np.ascontiguousarray(
                np.stack([np.asarray(gamma2, f32)[u],
                          np.asarray(beta2, f32)[u]], axis=1)),
            "m1T": np.ascontiguousarray(np.asarray(mask1, f32).T[u]),
            "m2T": np.ascontiguousarray(np.asarray(mask2, f32).T[u]),
        })
    return in_maps, np.asarray(b_out, f32)


def kernel(**inputs):
    global LAST_EXEC_NS
    if "nc" not in _CACHE:
        _CACHE["nc"] = build_bass()
    nc = _CACHE["nc"]

    in_maps, b_out = _prep_inputs(**inputs)
    trace = os.environ.get("TRN_KERNEL_TRACE", "0") == "1"
    res = run_bass_kernel_spmd(nc, in_maps, core_ids=list(range(NCORE)),
                               trace=trace)
    LAST_EXEC_NS = res.exec_time_ns
    out = np.concatenate([np.asarray(res.results[c]["out"], np.float32)
                          for c in range(NCORE)], axis=1)
    if b_out.any():
        out = out + b_out[None, :]
    return out

